# revision 16
# baseline (speedup 1.0000x reference)
"""Multi-head attention (B=2, S=2048, D=1024, H=16) on 8 TRN2 NeuronCores.

Sharding: data-parallel over batch (2) x tensor-parallel over head groups
(4 groups of 4 heads).  Core c = (b = c // 4, g = c % 4).  Each core:
  q/k/v = x[b] @ W{q,k,v}[:, 256g:256g+256] + b{q,k,v}[...]   (1/sqrt(dh)
  folded into Wq/bq on host), per-head softmax(q k^T) v, then a partial
  out-projection y_c = attn_out @ Wo[256g:256g+256, :].  Host sums the 4
  partials per batch and adds bo.

v2: all matmul operands bf16 (fp32r streams ~2.5x slower + no FWL),
head-PAIR score matmuls packed into disjoint PE row groups (K=64 each,
concurrent via tile_position), softmax denominators via
reciprocal_approx_fast, q/k bias adds on DVE instead of ACT (ACT is
saturated by EXP), emission order overlaps next q-half's scores with the
current normalization/out-projection.

Device layouts (per core):
  xT   [1024, 2048] bf16 (x[b] transposed on host)
  qT/kT: [256, 2048] as 2 sbuf tiles [128, 2048] (head h -> partitions
         64*(h%2).. of tile h//2)
  v_ext: 16 tiles [128, 260]; head h at cols 65h..65h+63, ones at 65h+64
         (ones column makes P @ V_ext also emit softmax denominators)
  scores^T per (pair, kt, qb): 2x [128, 512] psum (head a rows 0-63 of PE,
         head b rows 64-127, packed concurrently)
  attn_out^T: 2 tiles [128, 2048] bf16 (same head layout as qT)
  yT   [1024, 2048] f32 partial output (ExternalOutput)
"""

import os
import sys
import types
from contextlib import ExitStack

import numpy as np

D = 1024
S = 2048
C = 256          # head cols per core (4 heads x 64)
DH = 64
NH = 4           # heads per core
QH = 1024        # q-half size

# DVE-exp: scores are computed as u = s/16 (folded into Wq/bq on host).
# ACT recovers exp(s) via Exp(u*16); DVE tiles use p(u)^16 with a deg-4
# minimax p ~= e^u on [-0.58, 0.58] (c0 = 1); rel err ~6e-4 after ^16.
USE_DVE_EXP = True
DVE_EXP_MOD = 8       # slots with (idx % DVE_EXP_MOD) < DVE_EXP_TAKE go to DVE
DVE_EXP_TAKE = 3
EXP_C = (0.99976993, 0.50024631, 0.16994333, 0.04119434)

_CACHE = {}


def _register_dve_exp_ops():
    """Register the two custom DVE ops (idempotent). Returns (poly, pow16)."""
    import numpy as np
    from concourse import dve_ops
    from concourse.dve_spec import (
        Spec, Src0, C0, C1, C2, One, lower, _spill_c3_to_src1, sq, _has_src1)
    from concourse.dve_ops import DveOp
    from concourse.dve_uop import DveOpSpec

    if "EXP16_POLY" in dve_ops._SUB_OPCODE_FOR_NAME:
        by_name = {op.name: op for op in dve_ops.OPS}
        return by_name["EXP16_POLY"], by_name["POW16"]

    from concourse.dve_spec import C3
    body = One + Src0 * (C0 + Src0 * (C1 + Src0 * (C2 + Src0 * C3)))

    def ref_poly(in0, in1, c0, c1, c2):
        c3 = in1
        return (1.0 + in0 * (c0 + in0 * (c1 + in0 * (c2 + in0 * c3)))
                ).astype(np.float32)

    spec1 = Spec(body=_spill_c3_to_src1(body), reference=ref_poly)
    spec2 = Spec(
        body=sq(sq(sq(sq(Src0)))),
        reference=lambda in0, in1, c0, c1, c2:
            (in0.astype(np.float64) ** 16).astype(np.float32))

    ops = []
    for i, (name, spec) in enumerate(
            (("EXP16_POLY", spec1), ("POW16", spec2))):
        opcode = dve_ops._CUSTOM_DVE_ROW_BASE + len(dve_ops.OPS)
        shas = {}
        for ver in ("v3", "v4"):
            s = DveOpSpec(name=name, opcode=opcode,
                          uops=lower(spec, ver=ver), rd1_en=_has_src1(spec))
            shas[ver] = s.sha(ver)
        op = DveOp(name, spec, subdim=False, uops_sha=shas)
        dve_ops.OPS.append(op)
        dve_ops._SUB_OPCODE_FOR_NAME[name] = opcode
        dve_ops.CUSTOM_DVE_SPECS[name] = spec
        ops.append(op)
    return tuple(ops)


def _install_ntff_shim():
    try:
        import antenv.axon_hooks  # noqa: F401
        return
    except ImportError:
        pass
    try:
        from trn_agent_boot.trn_boot import _ntff_profile_via_ctypes
        hook = _ntff_profile_via_ctypes('/opt/axon/libaxon_pjrt.so')
    except Exception:
        hook = None
    mod = types.ModuleType('antenv.axon_hooks')
    mod.get_axon_ntff_profile_hook = lambda: hook
    mod.set_axon_ntff_profile_hook = lambda h: None
    sys.modules['antenv.axon_hooks'] = mod


def build_nc(seq=S, debug_dump=False):
    import concourse.bacc as bacc
    import concourse.mybir as mybir
    import concourse.tile as tile
    from concourse.bass import ts, ds

    F32 = mybir.dt.float32
    BF16 = mybir.dt.bfloat16
    ACT = mybir.ActivationFunctionType

    nqb = seq // 512          # 512-wide column blocks of seq
    nst = seq // 128          # 128-row tiles of seq
    qh_w = min(QH, seq)       # q-half width
    nqh = seq // qh_w         # number of q halves
    qh_b = qh_w // 512        # 512-blocks per q half

    nc = bacc.Bacc("TRN2", target_bir_lowering=False, debug=False)
    xT = nc.dram_tensor("xT", [D, seq], BF16, kind="ExternalInput")
    wq = nc.dram_tensor("wq", [D, C], BF16, kind="ExternalInput")
    wk = nc.dram_tensor("wk", [D, C], BF16, kind="ExternalInput")
    wv = nc.dram_tensor("wv", [D, C], BF16, kind="ExternalInput")
    wo = nc.dram_tensor("wo", [C, D], BF16, kind="ExternalInput")
    bqk = nc.dram_tensor("bqk", [128, 4], F32, kind="ExternalInput")  # [bq0 bq1 bk0 bk1]
    bv = nc.dram_tensor("bv", [1, C], F32, kind="ExternalInput")
    yT = nc.dram_tensor("yT", [D, seq], F32, kind="ExternalOutput")

    if debug_dump:
        qdbg = nc.dram_tensor("qdbg", [128, seq], BF16, kind="ExternalOutput")
        kdbg = nc.dram_tensor("kdbg", [128, seq], BF16, kind="ExternalOutput")
        vdbg = nc.dram_tensor("vdbg", [128, NH * 65], BF16, kind="ExternalOutput")
        adbg = nc.dram_tensor("adbg", [128, seq], BF16, kind="ExternalOutput")
        scdbg = nc.dram_tensor("scdbg", [128, 1024], F32, kind="ExternalOutput")
        ptdbg = nc.dram_tensor("ptdbg", [128, 1024], BF16, kind="ExternalOutput")
        pvdbg = nc.dram_tensor("pvdbg", [65, QH], F32, kind="ExternalOutput")
        rcdbg = nc.dram_tensor("rcdbg", [1, QH], F32, kind="ExternalOutput")

    with tile.TileContext(nc) as tc, ExitStack() as ctx:
        consts = ctx.enter_context(tc.tile_pool(name="consts", bufs=1))
        sbw = ctx.enter_context(tc.tile_pool(name="weights", bufs=1))
        sbx = ctx.enter_context(tc.tile_pool(name="xT", bufs=1))
        sbqkv = ctx.enter_context(tc.tile_pool(name="qkv", bufs=1))
        sbpt = ctx.enter_context(tc.tile_pool(name="pt", bufs=3))
        sbnrm = ctx.enter_context(tc.tile_pool(name="nrm", bufs=2))
        sby = ctx.enter_context(tc.tile_pool(name="ysb", bufs=4))

        # ---- constants ----
        bqk_sb = consts.tile([128, 4], F32, tag="bqk", name="bqk_sb")
        nc.sync.dma_start(bqk_sb[:], bqk[:, :])
        bv_row = consts.tile([1, C], F32, tag="bvrow", name="bv_row")
        nc.sync.dma_start(bv_row[:], bv[:, :])
        bvb = consts.tile([128, C], F32, tag="bvb", name="bvb")
        nc.gpsimd.partition_broadcast(bvb[:], bv_row[:])
        ones4 = consts.tile([128, NH], BF16, tag="ones4", name="ones4")
        nc.vector.memset(ones4[:], 1.0)
        if debug_dump:
            rtdbg = nc.dram_tensor("rtdbg", [1, 64], F32, kind="ExternalOutput")
            rt_in = consts.tile([1, 64], F32, tag="rtin", name="rtin")
            nc.vector.memset(rt_in[:], 3.0)
            rt_out = consts.tile([1, 64], F32, tag="rtout", name="rtout")
            nc.vector.reciprocal_approx_fast(rt_out[:], rt_in[:])
            nc.sync.dma_start(rtdbg[:, :], rt_out[:])

        # ---- loads: interleave weights with xT column blocks so the first
        #      projection group (wq + xt nb0) lands early ----
        xt_sb = [sbx.tile([128, seq], BF16, tag=f"xt{i}", name=f"xt{i}")
                 for i in range(8)]
        w_sb = {}
        for name, dram in (("q", wq), ("k", wk), ("v", wv)):
            tiles = []
            for i in range(8):
                t = sbw.tile([128, C], BF16, tag=f"w{name}{i}", name=f"w{name}{i}")
                nc.sync.dma_start(t[:], dram[ts(i, 128), :])
                tiles.append(t)
            w_sb[name] = tiles
            nbs = {"q": list(range(nqb))[:1], "k": list(range(nqb))[1:2],
                   "v": list(range(nqb))[2:]}[name]
            for nb in nbs:
                for i in range(8):
                    nc.sync.dma_start(xt_sb[i][:, ts(nb, 512)],
                                      xT[ts(i, 128), ts(nb, 512)])
        wo_sb = []
        for i in range(2):
            t = sbw.tile([128, D], BF16, tag=f"wo{i}", name=f"wo{i}")
            nc.sync.dma_start(t[:], wo[ts(i, 128), :])
            wo_sb.append(t)

        # ---- persistent activations ----
        qT_sb = [sbqkv.tile([128, seq], BF16, tag=f"qT{i}", name=f"qT{i}") for i in range(2)]
        kT_sb = [sbqkv.tile([128, seq], BF16, tag=f"kT{i}", name=f"kT{i}") for i in range(2)]
        v_sb = [sbqkv.tile([128, NH * 65], BF16, tag=f"v{i}", name=f"v{i}") for i in range(nst)]
        aT_sb = [sbqkv.tile([128, seq], BF16, tag=f"aT{i}", name=f"aT{i}") for i in range(2)]

        # ---- projections: qT, kT (bias added on DVE; ACT stays free for EXP) ----
        def proj_qk(psp, name, bias_col, mt):
            dst = qT_sb if name == "q" else kT_sb
            for nb in range(nqb):
                ps = psp.tile([128, 512], F32, tag="pp", name="pp")
                for kt in range(8):
                    nc.tensor.matmul(
                        ps[:],
                        lhsT=w_sb[name][kt][:, ts(mt, 128)],
                        rhs=xt_sb[kt][:, ts(nb, 512)],
                        start=(kt == 0), stop=(kt == 7),
                    )
                nc.vector.tensor_scalar_add(
                    dst[mt][:, ts(nb, 512)], ps[:],
                    bqk_sb[:, bias_col + mt:bias_col + mt + 1],
                )

        def proj_v(psp):
            for st in range(nst):
                ps = psp.tile([128, C], F32, tag="vp", name="vps")
                for kt in range(8):
                    nc.tensor.matmul(
                        ps[:],
                        lhsT=xt_sb[kt][:, ts(st, 128)],
                        rhs=w_sb["v"][kt][:],
                        start=(kt == 0), stop=(kt == 7),
                    )
                v3 = v_sb[st][:].rearrange("p (h e) -> p h e", e=65)
                nc.vector.tensor_copy(
                    v3[:, :, 64:65],
                    ones4[:].rearrange("p (h e) -> p h e", e=1))
                nc.vector.tensor_add(
                    v3[:, :, 0:64],
                    ps[:].rearrange("p (h e) -> p h e", e=64),
                    bvb[:].rearrange("p (h e) -> p h e", e=64),
                )

        # ---- attention: one head PAIR (2 heads sharing a qT/kT tile) ----
        # Scores for the two heads are issued back-to-back into disjoint PE
        # row groups (rows 0-63 / 64-127 via base partitions) so they stream
        # concurrently.  PV matmuls are full-K (128) and run serially.
        def attn_pair(scp, pvp, qh, p):
            qt, ktt = qT_sb[p], kT_sb[p]
            pv = {}
            for e in range(2):
                pv[e] = pvp.tile([65, qh_w], F32, tag=f"pv{e}", name=f"pv{e}")
            for kt in range(nst):
                for qb in range(qh_b):
                    sc, pt = {}, {}
                    for e in range(2):
                        sc[e] = scp.tile([128, 512], F32, tag="sc", name="sc")
                    for e in range(2):
                        po = 64 * e
                        nc.tensor.matmul(
                            sc[e][:],
                            lhsT=ktt[po:po + 64, ts(kt, 128)],
                            rhs=qt[po:po + 64, ds(qh * qh_w + qb * 512, 512)],
                            start=True, stop=True,
                        )
                    for e in range(2):
                        pt[e] = sbpt.tile([128, 512], BF16, tag=f"pt{e}", name=f"pt{e}")
                        nc.scalar.activation(pt[e][:], sc[e][:], ACT.Exp)
                    if debug_dump and qh == 0 and p == 0 and kt == 0 and qb == 0:
                        for e in range(2):
                            dsc = sby.tile([128, 512], F32, tag="dbgsc", name="dbgsc")
                            nc.vector.tensor_copy(dsc[:], sc[e][:])
                            nc.sync.dma_start(scdbg[:, ts(e, 512)], dsc[:])
                            nc.sync.dma_start(ptdbg[:, ts(e, 512)], pt[e][:])
                    for e in range(2):
                        h = 2 * p + e
                        nc.tensor.matmul(
                            pv[e][:, ts(qb, 512)],
                            lhsT=v_sb[kt][:, ds(65 * h, 65)],
                            rhs=pt[e][:],
                            start=(kt == 0), stop=(kt == nst - 1),
                        )
            return pv

        def norm_head(qh, p, e, pv):
            # denominator row PSUM -> SBUF via DMA (gpsimd + custom DVE ops
            # cannot access PSUM), broadcast to 64 partitions on gpsimd,
            # approx-reciprocal on 64 lanes, then normalize.
            drow = sbnrm.tile([1, qh_w], F32, tag="drow", name="drow")
            nc.vector.tensor_copy(drow[:], pv[64:65, :])
            dbc = sbnrm.tile([64, qh_w], F32, tag="dbc", name="dbc")
            nc.gpsimd.partition_broadcast(dbc[:], drow[:])
            rbc = sbnrm.tile([64, qh_w], F32, tag="rbc", name="rbc")
            nc.vector.reciprocal_approx_fast(rbc[:], dbc[:])
            if debug_dump and qh == 0 and p == 0 and e == 0:
                dpv = sby.tile([65, qh_w], F32, tag="dbgpv", name="dbgpv")
                nc.vector.tensor_copy(dpv[:], pv[:])
                nc.sync.dma_start(pvdbg[:, :], dpv[:])
                nc.sync.dma_start(rcdbg[:, :], rbc[0:1, :])
            nc.vector.tensor_mul(
                aT_sb[p][64 * e:64 * e + 64, ds(qh * qh_w, qh_w)],
                pv[0:64, :], rbc[:],
            )

        def out_proj(ypp, qh):
            for mt in range(8):
                for nb in range(qh * qh_b, (qh + 1) * qh_b):
                    yp = ypp.tile([128, 512], F32, tag="yp", name="yp")
                    for kt2 in range(2):
                        nc.tensor.matmul(
                            yp[:],
                            lhsT=wo_sb[kt2][:, ts(mt, 128)],
                            rhs=aT_sb[kt2][:, ts(nb, 512)],
                            start=(kt2 == 0), stop=(kt2 == 1),
                        )
                    yt = sby.tile([128, 512], F32, tag="yt", name="yt")
                    nc.vector.tensor_copy(yt[:], yp[:])
                    nc.sync.dma_start(yT[ts(mt, 128), ts(nb, 512)], yt[:])

        with tc.tile_pool(name="pproj", bufs=3, space="PSUM") as psp:
            proj_qk(psp, "q", 0, 0)
            proj_qk(psp, "k", 2, 0)
            proj_v(psp)
            proj_qk(psp, "q", 0, 1)
            proj_qk(psp, "k", 2, 1)
        with (
            tc.tile_pool(name="psc", bufs=2, space="PSUM") as scp,
            tc.tile_pool(name="ppv", bufs=1, space="PSUM") as pvp,
            tc.tile_pool(name="pyp", bufs=2, space="PSUM") as ypp,
        ):
            # Interleave: emit out_proj(qh) after the NEXT q-half's first
            # pair so the PE has score work during qh's final normalization.
            work = [(qh, p) for qh in range(nqh) for p in range(2)]
            for i, (qh, p) in enumerate(work):
                pv = attn_pair(scp, pvp, qh, p)
                for e in range(2):
                    norm_head(qh, p, e, pv[e])
                if p == 1 and qh + 1 < nqh:
                    # defer this qh's out_proj past the next pair's scores
                    continue
                if p == 0 and qh > 0:
                    out_proj(ypp, qh - 1)
            out_proj(ypp, nqh - 1)
            if debug_dump:
                nc.sync.dma_start(qdbg[:, :], qT_sb[0][:])
                nc.sync.dma_start(kdbg[:, :], kT_sb[0][:])
                nc.sync.dma_start(vdbg[:, :], v_sb[0][:])
                nc.sync.dma_start(adbg[:, :], aT_sb[0][:])

    nc.compile()
    return nc


def make_in_maps(x, Wq, bq, Wk, bk, Wv, bv, Wo):
    """Shard full inputs into 8 per-core input maps."""
    import ml_dtypes
    BF = ml_dtypes.bfloat16
    scale = np.float32(1.0 / np.sqrt(DH))
    xT = [np.ascontiguousarray(x[b].T).astype(BF) for b in range(2)]
    in_maps = []
    for c in range(8):
        b, g = c // 4, c % 4
        sl = slice(C * g, C * (g + 1))
        bq_g = (bq[sl] * scale).reshape(2, 128).T
        bk_g = bk[sl].reshape(2, 128).T
        in_maps.append({
            "xT": xT[b],
            "wq": (np.ascontiguousarray(Wq[:, sl]) * scale).astype(BF),
            "wk": np.ascontiguousarray(Wk[:, sl]).astype(BF),
            "wv": np.ascontiguousarray(Wv[:, sl]).astype(BF),
            "wo": np.ascontiguousarray(Wo[sl, :]).astype(BF),
            "bqk": np.ascontiguousarray(
                np.concatenate([bq_g, bk_g], axis=1)).astype(np.float32),
            "bv": bv[sl].reshape(1, C).astype(np.float32),
        })
    return in_maps


def kernel(x, Wq, bq, Wk, bk, Wv, bv, Wo, bo):
    if os.environ.get("JAX_PLATFORMS") and \
            "axon" not in os.environ["JAX_PLATFORMS"]:
        os.environ.pop("JAX_PLATFORMS")
    trace = bool(os.environ.get("KERNEL_TRACE"))
    if trace:
        _install_ntff_shim()
    from concourse import bass_utils

    x = np.asarray(x, dtype=np.float32)
    in_maps = make_in_maps(
        x, np.asarray(Wq), np.asarray(bq), np.asarray(Wk), np.asarray(bk),
        np.asarray(Wv), np.asarray(bv), np.asarray(Wo))

    if "nc" not in _CACHE:
        _CACHE["nc"] = build_nc()
    res = bass_utils.run_bass_kernel_spmd(
        _CACHE["nc"], in_maps, core_ids=list(range(8)), trace=trace)
    _CACHE["exec_time_ns"] = res.exec_time_ns

    bo = np.asarray(bo, dtype=np.float32)
    out = np.empty((2, S, D), dtype=np.float32)
    for b in range(2):
        acc = res.results[4 * b]["yT"].copy()
        for g in range(1, 4):
            acc += res.results[4 * b + g]["yT"]
        out[b] = acc.T + bo
    return out


# revision 25
# speedup vs baseline: 1.2511x; 1.2511x over previous
"""Multi-head attention (B=2, S=2048, D=1024, H=16) on 8 TRN2 NeuronCores.

Sharding: data-parallel over batch (2) x tensor-parallel over head groups
(4 groups of 4 heads).  Core c = (b = c // 4, g = c % 4).  Each core:
  q/k/v = x[b] @ W{q,k,v}[:, 256g:256g+256] + b{q,k,v}[...]   (1/sqrt(dh)
  folded into Wq/bq on host), per-head softmax(q k^T) v, then a partial
  out-projection y_c = attn_out @ Wo[256g:256g+256, :].  Host sums the 4
  partials per batch and adds bo.

v4: all matmul operands bf16.  The kernel is ACT(EXP)-bound (~147us of
exp at [128,1024] tiles), so every other PE op (v/q1/k1 projections and
the qh0 out-projection) is interleaved into the attention loop as filler
to keep the PE dense — scattered PE micro-idle lets the HAM clock gate
re-throttle the PE to 1.2 GHz (measured: half-speed matmuls for ~190us
of the span in the phase-separated version).

Device layouts (per core):
  xT   [1024, 2048] bf16 (x[b] transposed on host)
  qT/kT: [256, 2048] as 2 sbuf tiles [128, 2048] (head h -> partitions
         64*(h%2).. of tile h//2)
  v_ext: 16 tiles [128, 260]; head h at cols 65h..65h+63, ones at 65h+64
         (ones column makes P @ V_ext also emit softmax denominators)
  scores^T per (head, kt): [128, 1024] psum (one q-half), exp'd in one
         ACT instruction to amortize the 352-cycle ACT startup
  attn_out^T: 2 tiles [128, 2048] bf16 (same head layout as qT)
  yT   [1024, 2048] f32 partial output (ExternalOutput)

Softmax normalization (PSUM is only DVE/ACT-readable; gpsimd and custom
DVE ops are SBUF-only): DVE copies the denominator row out of PSUM,
gpsimd broadcasts it to 64 partitions, DVE reciprocal_approx_fast (~51
ULP custom op), DVE multiply.
"""

import os
import sys
import types
from contextlib import ExitStack

import numpy as np

D = 1024
S = 2048
C = 256          # head cols per core (4 heads x 64)
DH = 64
NH = 4           # heads per core
QH = 1024        # q-half size

_CACHE = {}


def _install_ntff_shim():
    try:
        import antenv.axon_hooks  # noqa: F401
        return
    except ImportError:
        pass
    try:
        from trn_agent_boot.trn_boot import _ntff_profile_via_ctypes
        hook = _ntff_profile_via_ctypes('/opt/axon/libaxon_pjrt.so')
    except Exception:
        hook = None
    mod = types.ModuleType('antenv.axon_hooks')
    mod.get_axon_ntff_profile_hook = lambda: hook
    mod.set_axon_ntff_profile_hook = lambda h: None
    sys.modules['antenv.axon_hooks'] = mod


def build_nc(seq=S, debug_dump=False):
    import concourse.bacc as bacc
    import concourse.mybir as mybir
    import concourse.tile as tile
    from concourse.bass import ts, ds

    F32 = mybir.dt.float32
    BF16 = mybir.dt.bfloat16
    ACT = mybir.ActivationFunctionType

    nqb = seq // 512          # 512-wide column blocks of seq
    nst = seq // 128          # 128-row tiles of seq
    qh_w = min(QH, seq)       # q-half width
    nqh = seq // qh_w         # number of q halves
    qh_b = qh_w // 512        # 512-blocks per q half

    nc = bacc.Bacc("TRN2", target_bir_lowering=False, debug=False)
    xT = nc.dram_tensor("xT", [D, seq], BF16, kind="ExternalInput")
    wq = nc.dram_tensor("wq", [D, C], BF16, kind="ExternalInput")
    wk = nc.dram_tensor("wk", [D, C], BF16, kind="ExternalInput")
    wv = nc.dram_tensor("wv", [D, C], BF16, kind="ExternalInput")
    wo = nc.dram_tensor("wo", [C, D], BF16, kind="ExternalInput")
    bqk = nc.dram_tensor("bqk", [128, 4], F32, kind="ExternalInput")  # [bq0 bq1 bk0 bk1]
    bv = nc.dram_tensor("bv", [1, C], F32, kind="ExternalInput")
    yT = nc.dram_tensor("yT", [D, seq], F32, kind="ExternalOutput")

    if debug_dump:
        qdbg = nc.dram_tensor("qdbg", [128, seq], BF16, kind="ExternalOutput")
        adbg = nc.dram_tensor("adbg", [128, seq], BF16, kind="ExternalOutput")

    with tile.TileContext(nc) as tc, ExitStack() as ctx:
        consts = ctx.enter_context(tc.tile_pool(name="consts", bufs=1))
        sbw = ctx.enter_context(tc.tile_pool(name="weights", bufs=1))
        sbx = ctx.enter_context(tc.tile_pool(name="xT", bufs=1))
        sbqkv = ctx.enter_context(tc.tile_pool(name="qkv", bufs=1))
        sbpt = ctx.enter_context(tc.tile_pool(name="pt", bufs=3))
        sbnrm = ctx.enter_context(tc.tile_pool(name="nrm", bufs=2))
        sby = ctx.enter_context(tc.tile_pool(name="ysb", bufs=4))

        # ---- constants ----
        bqk_sb = consts.tile([128, 4], F32, tag="bqk", name="bqk_sb")
        nc.sync.dma_start(bqk_sb[:], bqk[:, :])
        bv_row = consts.tile([1, C], F32, tag="bvrow", name="bv_row")
        nc.sync.dma_start(bv_row[:], bv[:, :])
        bvb = consts.tile([128, C], F32, tag="bvb", name="bvb")
        nc.gpsimd.partition_broadcast(bvb[:], bv_row[:])
        ones4 = consts.tile([128, NH], BF16, tag="ones4", name="ones4")
        nc.vector.memset(ones4[:], 1.0)

        # ---- loads: interleave weights with xT column blocks ----
        xt_sb = [sbx.tile([128, seq], BF16, tag=f"xt{i}", name=f"xt{i}")
                 for i in range(8)]
        w_sb = {}
        for name, dram in (("q", wq), ("k", wk), ("v", wv)):
            tiles = []
            for i in range(8):
                t = sbw.tile([128, C], BF16, tag=f"w{name}{i}", name=f"w{name}{i}")
                nc.sync.dma_start(t[:], dram[ts(i, 128), :])
                tiles.append(t)
            w_sb[name] = tiles
            nbs = {"q": list(range(nqb))[:1], "k": list(range(nqb))[1:2],
                   "v": list(range(nqb))[2:]}[name]
            for nb in nbs:
                for i in range(8):
                    nc.sync.dma_start(xt_sb[i][:, ts(nb, 512)],
                                      xT[ts(i, 128), ts(nb, 512)])
        wo_sb = []
        for i in range(2):
            t = sbw.tile([128, D], BF16, tag=f"wo{i}", name=f"wo{i}")
            nc.sync.dma_start(t[:], wo[ts(i, 128), :])
            wo_sb.append(t)

        # ---- persistent activations ----
        qT_sb = [sbqkv.tile([128, seq], BF16, tag=f"qT{i}", name=f"qT{i}") for i in range(2)]
        kT_sb = [sbqkv.tile([128, seq], BF16, tag=f"kT{i}", name=f"kT{i}") for i in range(2)]
        v_sb = [sbqkv.tile([128, NH * 65], BF16, tag=f"v{i}", name=f"v{i}") for i in range(nst)]
        aT_sb = [sbqkv.tile([128, seq], BF16, tag=f"aT{i}", name=f"aT{i}") for i in range(2)]

        # ---- PE filler generator: yields callables emitting ~1-2us of PE
        #      work each; threaded into the attention loop to keep the PE
        #      dense while ACT (the bottleneck) drains EXPs.
        def proj_qk_block(psp, name, mt, nb):
            dst = (qT_sb if name == "q" else kT_sb)[mt]
            bias_col = (0 if name == "q" else 2) + mt
            ps = psp.tile([128, 512], F32, tag="aux", name="pp")
            for kt in range(8):
                nc.tensor.matmul(
                    ps[:],
                    lhsT=w_sb[name][kt][:, ts(mt, 128)],
                    rhs=xt_sb[kt][:, ts(nb, 512)],
                    start=(kt == 0), stop=(kt == 7),
                )
            nc.vector.tensor_scalar_add(
                dst[:, ts(nb, 512)], ps[:],
                bqk_sb[:, bias_col:bias_col + 1],
            )

        def proj_v_block(psp, st):
            ps = psp.tile([128, 512], F32, tag="aux", name="vps")
            for kt in range(8):
                nc.tensor.matmul(
                    ps[:, 0:C],
                    lhsT=xt_sb[kt][:, ts(st, 128)],
                    rhs=w_sb["v"][kt][:],
                    start=(kt == 0), stop=(kt == 7),
                )
            v3 = v_sb[st][:].rearrange("p (h e) -> p h e", e=65)
            nc.vector.tensor_copy(
                v3[:, :, 64:65],
                ones4[:].rearrange("p (h e) -> p h e", e=1))
            nc.vector.tensor_add(
                v3[:, :, 0:64],
                ps[:, 0:C].rearrange("p (h e) -> p h e", e=64),
                bvb[:].rearrange("p (h e) -> p h e", e=64),
            )

        def out_proj_block(ypp, qh, mt, nb):
            yp = ypp.tile([128, 512], F32, tag="aux", name="yp")
            for kt2 in range(2):
                nc.tensor.matmul(
                    yp[:],
                    lhsT=wo_sb[kt2][:, ts(mt, 128)],
                    rhs=aT_sb[kt2][:, ts(nb, 512)],
                    start=(kt2 == 0), stop=(kt2 == 1),
                )
            yt = sby.tile([128, 512], F32, tag="yt", name="yt")
            nc.vector.tensor_copy(yt[:], yp[:])
            nc.sync.dma_start(yT[ts(mt, 128), ts(nb, 512)], yt[:])

        def norm_head(qh, p, e, pv):
            drow = sbnrm.tile([1, qh_w], F32, tag="drow", name="drow")
            nc.vector.tensor_copy(drow[:], pv[64:65, :])
            dbc = sbnrm.tile([64, qh_w], F32, tag="dbc", name="dbc")
            nc.gpsimd.partition_broadcast(dbc[:], drow[:])
            rbc = sbnrm.tile([64, qh_w], F32, tag="rbc", name="rbc")
            nc.vector.reciprocal_approx_fast(rbc[:], dbc[:])
            nc.vector.tensor_mul(
                aT_sb[p][64 * e:64 * e + 64, ds(qh * qh_w, qh_w)],
                pv[0:64, :], rbc[:],
            )

        with (
            tc.tile_pool(name="psc", bufs=2, space="PSUM") as scp,
            tc.tile_pool(name="ppv", bufs=1, space="PSUM") as pvp,
            tc.tile_pool(name="paux", bufs=2, space="PSUM") as auxp,
        ):
            # Lead-in: q0 + k0 projections (PE-dense; also warms the HAM).
            for nb in range(nqb):
                proj_qk_block(auxp, "q", 0, nb)
            for nb in range(nqb):
                proj_qk_block(auxp, "k", 0, nb)

            # Filler queue (consumed one block per odd kt slot): q1/k1
            # projection blocks, then the qh0 out-projection once all qh0
            # heads are normed.  v projections are emitted INLINE in the
            # first unit (PV(kt) consumes v_sb[kt] just-in-time, and the
            # PE queue is in-order — a later filler can't satisfy an
            # earlier PV's dependency).
            filler = []
            for nb in range(nqb):
                filler.append(("q1", nb))
                filler.append(("k1", nb))

            def emit_filler(n):
                for _ in range(n):
                    if not filler:
                        return
                    kind, a = filler.pop(0)
                    if kind == "q1":
                        proj_qk_block(auxp, "q", 1, a)
                    elif kind == "k1":
                        proj_qk_block(auxp, "k", 1, a)
                    elif kind == "op":
                        out_proj_block(auxp, a[0], a[1], a[2])

            # Attention: one head at a time; per (qh, head): nst kt slots.
            # Each slot: 2 score matmuls [128,512] -> one EXP [128,1024]
            # -> 2 PV matmuls; plus ~1 filler block to keep PE dense.
            # head order: tile0 heads (0,1) then tile1 heads (2,3) per qh
            units = [(qh, h) for qh in range(nqh) for h in range(NH)]
            for ui, (qh, h) in enumerate(units):
                p, e = h // 2, h % 2
                po = 64 * e
                qt, ktt = qT_sb[p], kT_sb[p]
                if (qh, h) == (0, 2):
                    # qT1/kT1 must exist before this unit's scores
                    emit_filler(sum(1 for k, _ in filler if k in ("q1", "k1")))
                pv = pvp.tile([65, qh_w], F32, tag="pv", name="pv")
                for kt in range(nst):
                    if ui == 0:
                        proj_v_block(auxp, kt)
                    sc = scp.tile([128, qh_w], F32, tag="sc", name="sc")
                    for qb in range(qh_b):
                        nc.tensor.matmul(
                            sc[:, ts(qb, 512)],
                            lhsT=ktt[po:po + 64, ts(kt, 128)],
                            rhs=qt[po:po + 64, ds(qh * qh_w + qb * 512, 512)],
                            start=True, stop=True,
                        )
                    pt = sbpt.tile([128, qh_w], BF16, tag="pt", name="pt")
                    nc.scalar.activation(pt[:], sc[:], ACT.Exp)
                    for qb in range(qh_b):
                        nc.tensor.matmul(
                            pv[:, ts(qb, 512)],
                            lhsT=v_sb[kt][:, ds(65 * h, 65)],
                            rhs=pt[:, ts(qb, 512)],
                            start=(kt == 0), stop=(kt == nst - 1),
                        )
                    if ui > 0 and kt % 2 == 1:
                        emit_filler(1)
                norm_head(qh, p, e, pv)
                if qh == 0 and h == NH - 1 and nqh > 1:
                    # qh0 fully normed: queue its out-projection as filler
                    for mt in range(8):
                        for nb in range(qh_b):
                            filler.append(("op", (0, mt, nb)))
            # drain remaining filler (qh0 out-proj tail if any)
            emit_filler(len(filler))
            # final q-half out-projection
            for mt in range(8):
                for nb in range((nqh - 1) * qh_b, nqh * qh_b):
                    out_proj_block(auxp, nqh - 1, mt, nb)

            if debug_dump:
                nc.sync.dma_start(qdbg[:, :], qT_sb[0][:])
                nc.sync.dma_start(adbg[:, :], aT_sb[0][:])

    nc.compile()
    return nc


def make_in_maps(x, Wq, bq, Wk, bk, Wv, bv, Wo):
    """Shard full inputs into 8 per-core input maps."""
    import ml_dtypes
    BF = ml_dtypes.bfloat16
    scale = np.float32(1.0 / np.sqrt(DH))
    xT = [np.ascontiguousarray(x[b].T).astype(BF) for b in range(2)]
    in_maps = []
    for c in range(8):
        b, g = c // 4, c % 4
        sl = slice(C * g, C * (g + 1))
        bq_g = (bq[sl] * scale).reshape(2, 128).T
        bk_g = bk[sl].reshape(2, 128).T
        in_maps.append({
            "xT": xT[b],
            "wq": (np.ascontiguousarray(Wq[:, sl]) * scale).astype(BF),
            "wk": np.ascontiguousarray(Wk[:, sl]).astype(BF),
            "wv": np.ascontiguousarray(Wv[:, sl]).astype(BF),
            "wo": np.ascontiguousarray(Wo[sl, :]).astype(BF),
            "bqk": np.ascontiguousarray(
                np.concatenate([bq_g, bk_g], axis=1)).astype(np.float32),
            "bv": bv[sl].reshape(1, C).astype(np.float32),
        })
    return in_maps


def kernel(x, Wq, bq, Wk, bk, Wv, bv, Wo, bo):
    if os.environ.get("JAX_PLATFORMS") and \
            "axon" not in os.environ["JAX_PLATFORMS"]:
        os.environ.pop("JAX_PLATFORMS")
    trace = bool(os.environ.get("KERNEL_TRACE"))
    if trace:
        _install_ntff_shim()
    from concourse import bass_utils

    x = np.asarray(x, dtype=np.float32)
    in_maps = make_in_maps(
        x, np.asarray(Wq), np.asarray(bq), np.asarray(Wk), np.asarray(bk),
        np.asarray(Wv), np.asarray(bv), np.asarray(Wo))

    if "nc" not in _CACHE:
        _CACHE["nc"] = build_nc()
    res = bass_utils.run_bass_kernel_spmd(
        _CACHE["nc"], in_maps, core_ids=list(range(8)), trace=trace)
    _CACHE["exec_time_ns"] = res.exec_time_ns

    bo = np.asarray(bo, dtype=np.float32)
    out = np.empty((2, S, D), dtype=np.float32)
    for b in range(2):
        acc = res.results[4 * b]["yT"].copy()
        for g in range(1, 4):
            acc += res.results[4 * b + g]["yT"]
        out[b] = acc.T + bo
    return out
